# revision 1
# baseline (speedup 1.0000x reference)
"""GAT-stack (4-layer GraphEncoder/AttributeDecoder) Bass kernel for 8 trn2 cores.

Strategy (dst-sharded, uniform SPMD schedule):
  - nodes padded to NP = 8*NLOC; core c owns dst rows [c*NLOC, (c+1)*NLOC)
  - per layer each core computes a 256B-row bf16 "source table"
      row i = [1.0_bf16 | v_i (64 bf16) | pad | s_hat_i (fp32 bits in 2 bf16 slots)]
    and AllGathers it -> T_full.  v = h1 for layer 1, v = x (prev output) for
    layers 2-4 (aggregate in input space, apply W after aggregation).
  - d_hat (dst-side attention term) stays core-local in a 256B-row f32 table.
  - edges are packed host-side into 128-edge groups assigned to
    (dst-tile t, src-chunk c) cells with core-uniform group counts, so the
    instruction stream is identical on all cores; per-core data lives in the
    int16 gather-index tensors and the dst_rel (onehot) tensor.
  - per group: dma_gather 128 table rows by src, per-edge
    p = exp(leaky_relu(s_hat+d_hat)), onehot(dst_rel)*p as bf16 lhsT,
    matmul-accumulate [128 dsts x (denom | 64 feats)] in PSUM.
  - evacuate per dst-tile; layers 2-4: transpose + matmul by W; all layers:
    divide by denom, bias, relu, (BN for 1-3).
"""

import numpy as np
import ml_dtypes


class Cfg:
    def __init__(self, N, E, F, H, NCORES, NLOC, TR, NCHUNK):
        self.N, self.E, self.F, self.H = N, E, F, H
        self.NCORES, self.NLOC = NCORES, NLOC
        self.NP = NCORES * NLOC
        self.NTILES = NLOC // 128
        self.TR = TR
        assert self.NTILES % TR == 0
        self.NRANGES = self.NTILES // TR
        self.NCHUNK = NCHUNK
        assert self.NP % NCHUNK == 0
        self.CHUNK = self.NP // NCHUNK
        assert self.CHUNK <= 32768
        self.SLOPE = 0.2
        self.EPS = 1e-5


FULL = Cfg(N=100000, E=1600000, F=256, H=64, NCORES=8, NLOC=12800, TR=4, NCHUNK=4)


def build_plan(edge_index, cfg):
    """Pack edges into the uniform schedule. Returns schedule + per-core data."""
    c_ = cfg
    src = np.concatenate([np.asarray(edge_index[0], np.int64),
                          np.arange(c_.N, dtype=np.int64)])
    dst = np.concatenate([np.asarray(edge_index[1], np.int64),
                          np.arange(c_.N, dtype=np.int64)])
    core = dst // c_.NLOC
    dst_local = dst % c_.NLOC
    tile = dst_local // 128
    chunk = src // c_.CHUNK

    cnt = np.zeros((c_.NCORES, c_.NTILES, c_.NCHUNK), dtype=np.int64)
    np.add.at(cnt, (core, tile, chunk), 1)
    gtc = np.maximum(np.ceil(cnt / 128).astype(np.int64).max(axis=0), 1)

    # flat group schedule: for r: for c: for t in range r: gtc[t,c] groups
    group_t, group_c = [], []
    for r in range(c_.NRANGES):
        for ch in range(c_.NCHUNK):
            for t in range(r * c_.TR, (r + 1) * c_.TR):
                group_t += [t] * int(gtc[t, ch])
                group_c += [ch] * int(gtc[t, ch])
    group_t = np.array(group_t, dtype=np.int64)
    group_c = np.array(group_c, dtype=np.int64)
    NG = len(group_t)
    NSLOT = NG * 128

    first_group = np.full((c_.NTILES, c_.NCHUNK), -1, dtype=np.int64)
    for g in range(NG):
        t, ch = group_t[g], group_c[g]
        if first_group[t, ch] < 0:
            first_group[t, ch] = g

    # piece structure: one (hs-gather, d-gather) pair per (range, chunk)
    pieces = []  # (slot0, nslots, chunk, group0, ngroups)
    pos_groups = 0
    for r in range(c_.NRANGES):
        for ch in range(c_.NCHUNK):
            ng = int(gtc[r * c_.TR:(r + 1) * c_.TR, ch].sum())
            pieces.append((pos_groups * 128, ng * 128, ch, pos_groups, ng))
            pos_groups += ng
    assert pos_groups == NG

    # per-core slot data
    idx_hs = np.zeros((c_.NCORES, NSLOT), dtype=np.int32)
    idx_d = np.zeros((c_.NCORES, NSLOT), dtype=np.int32)
    for cid in range(c_.NCORES):
        idx_d[cid, :] = cid * c_.NLOC
    dst_rel = np.full((c_.NCORES, NSLOT), 128, dtype=np.int32)
    for cid in range(c_.NCORES):
        m = core == cid
        s_src, s_dstl = src[m], dst_local[m]
        s_tile, s_chunk = s_dstl // 128, s_src // c_.CHUNK
        order = np.lexsort((s_dstl, s_chunk, s_tile))
        s_src, s_dstl, s_tile, s_chunk = (
            s_src[order], s_dstl[order], s_tile[order], s_chunk[order])
        keys = s_tile * c_.NCHUNK + s_chunk
        uniq, starts = np.unique(keys, return_index=True)
        starts = list(starts) + [len(keys)]
        for i, k in enumerate(uniq):
            t, ch = divmod(int(k), c_.NCHUNK)
            lo, hi = starts[i], starts[i + 1]
            n = hi - lo
            s0 = int(first_group[t, ch]) * 128
            assert n <= int(gtc[t, ch]) * 128
            idx_hs[cid, s0:s0 + n] = s_src[lo:hi].astype(np.int32)
            idx_d[cid, s0:s0 + n] = (cid * c_.NLOC + s_dstl[lo:hi]).astype(np.int32)
            dst_rel[cid, s0:s0 + n] = (s_dstl[lo:hi] % 128).astype(np.int32)

    return dict(group_t=group_t, group_c=group_c, gtc=gtc, NG=NG, NSLOT=NSLOT,
                pieces=pieces, idx_hs=idx_hs, idx_d=idx_d, dst_rel=dst_rel)


def _wrap16(a):
    """idx i -> [i%16, i//16], replicated to 128 partitions."""
    a = a.reshape(-1, 16).T  # [16, n/16]
    return np.tile(a, (8, 1)).copy()  # [128, n/16]


def build_host_inputs(plan, cfg, inputs):
    """Per-core in_maps for run_bass_kernel_spmd."""
    c_ = cfg
    feat = np.asarray(inputs["feat"], np.float32)
    featp = np.zeros((c_.NP, c_.F), np.float32)
    featp[:c_.N] = feat

    def rep(v, n=64):
        v = np.asarray(v, np.float32).reshape(1, -1)
        return np.tile(v, (128, 1)).copy()

    Ws = {i: np.asarray(inputs[f"W{i}"], np.float32) for i in range(1, 5)}
    common = {}
    common["W1"] = Ws[1]
    for i in (2, 3, 4):
        common[f"W{i}"] = Ws[i]
    # attention vectors: layer1 uses h-space (as1/ad1 directly);
    # layers 2-4 fold through W: cs = W @ as, cd = W @ ad
    common["cs1"] = rep(inputs["as1"])
    common["cd1"] = rep(inputs["ad1"])
    for i in (2, 3, 4):
        common[f"cs{i}"] = rep(Ws[i] @ np.asarray(inputs[f"as{i}"], np.float32))
        common[f"cd{i}"] = rep(Ws[i] @ np.asarray(inputs[f"ad{i}"], np.float32))
    for i in (1, 2, 3, 4):
        common[f"b{i}"] = rep(inputs[f"bi{i}"], None)
    for i in (1, 2, 3):
        sc = np.asarray(inputs[f"g{i}"], np.float32) / np.sqrt(1.0 + c_.EPS)
        common[f"scale{i}"] = rep(sc)
        common[f"shift{i}"] = rep(inputs[f"be{i}"])

    in_maps = []
    for cid in range(c_.NCORES):
        m = dict(common)
        sl = slice(cid * c_.NLOC, (cid + 1) * c_.NLOC)
        m["featT"] = np.ascontiguousarray(featp[sl].T)  # [F, NLOC]
        m["idx_hs"] = plan["idx_hs"][cid].reshape(-1, 128).T.copy()
        m["idx_d"] = plan["idx_d"][cid].reshape(-1, 128).T.copy()
        # dst_rel laid out [128, NG]: slot (p, g) -> value
        m["dst_rel"] = plan["dst_rel"][cid].reshape(-1, 128).T.astype(
            ml_dtypes.bfloat16).copy()
        in_maps.append(m)
    return in_maps


def build_bass(plan, cfg):
    import concourse.bass as bass
    import concourse.mybir as mybir
    import concourse.tile as tile
    from concourse import bacc
    from concourse.masks import make_identity

    c_ = cfg
    f32 = mybir.dt.float32
    bf16 = mybir.dt.bfloat16
    i16 = mybir.dt.int16
    AF = mybir.ActivationFunctionType
    ALU = mybir.AluOpType

    NG, NSLOT = plan["NG"], plan["NSLOT"]
    group_t, group_c, gtc = plan["group_t"], plan["group_c"], plan["gtc"]
    pieces = plan["pieces"]

    nc = bacc.Bacc("TRN2", num_devices=c_.NCORES)

    # ---- I/O ----
    featT = nc.dram_tensor("featT", [c_.F, c_.NLOC], f32, kind="ExternalInput")
    idx_hs_d = nc.dram_tensor("idx_hs", [128, NG], mybir.dt.int32, kind="ExternalInput")
    idx_d_d = nc.dram_tensor("idx_d", [128, NG], mybir.dt.int32, kind="ExternalInput")
    dst_rel_d = nc.dram_tensor("dst_rel", [128, NG], bf16, kind="ExternalInput")
    W_d = {1: nc.dram_tensor("W1", [c_.F, c_.H], f32, kind="ExternalInput")}
    for i in (2, 3, 4):
        dout = c_.F if i == 4 else c_.H
        W_d[i] = nc.dram_tensor(f"W{i}", [c_.H, dout], f32, kind="ExternalInput")
    small = {}
    for nm in ["cs1", "cd1", "cs2", "cd2", "cs3", "cd3", "cs4", "cd4",
               "b1", "b2", "b3", "scale1", "shift1", "scale2", "shift2",
               "scale3", "shift3"]:
        small[nm] = nc.dram_tensor(nm, [128, c_.H], f32, kind="ExternalInput")
    small["b4"] = nc.dram_tensor("b4", [128, c_.F], f32, kind="ExternalInput")
    out_d = nc.dram_tensor("out", [c_.NLOC, c_.F], f32, kind="ExternalOutput")

    # ---- internal DRAM ----
    T_local = nc.dram_tensor("T_local", [c_.NLOC, 128], bf16)
    T_full = nc.dram_tensor("T_full", [c_.NP, 128], bf16, addr_space="Shared")

    with tile.TileContext(nc) as tc:
        with (
            tc.tile_pool(name="const", bufs=1) as constp,
            tc.tile_pool(name="resid", bufs=1) as resid,
            tc.tile_pool(name="work", bufs=2) as work,
            tc.tile_pool(name="evac", bufs=3) as evacp,
            tc.tile_pool(name="psum", bufs=1, space="PSUM") as psum,
        ):
            # ---- constants ----
            ident = constp.tile([128, 128], f32)
            make_identity(nc, ident)
            iota_i = constp.tile([128, 128], mybir.dt.int32)
            nc.gpsimd.iota(iota_i[:], pattern=[[1, 128]], base=0,
                           channel_multiplier=0)
            iota_bf = constp.tile([128, 128], bf16)
            nc.vector.tensor_copy(iota_bf[:], iota_i[:])
            sm = {}
            for nm, dt_ in small.items():
                t = constp.tile(list(dt_.shape), f32, tag=f"sm_{nm}", name=f"sm_{nm}")
                nc.sync.dma_start(t[:], dt_[:])
                sm[nm] = t
            W_sb = {}
            for i in (2, 3, 4):
                dout = c_.F if i == 4 else c_.H
                W_sb[i] = constp.tile([c_.H, dout], f32, tag=f"W{i}", name=f"Wsb{i}")
                nc.sync.dma_start(W_sb[i][:], W_d[i][:])
            dst_rel_sb = constp.tile([128, NG], bf16)
            nc.sync.dma_start(dst_rel_sb[:], dst_rel_d[:])
            idx_hs_sb = constp.tile([128, NG], mybir.dt.int32)
            nc.sync.dma_start(idx_hs_sb[:], idx_hs_d[:])
            idx_d_sb = constp.tile([128, NG], mybir.dt.int32)
            nc.sync.dma_start(idx_d_sb[:], idx_d_d[:])

            # ---- residents ----
            x_sb = resid.tile([128, c_.NTILES, c_.H], f32)       # v for next layer
            den_sb = resid.tile([128, c_.NTILES], f32)           # 1/denom per dst
            T_sb = resid.tile([128, c_.NTILES, 128], bf16)       # table build buffer

            ones_bf = constp.tile([128, 1], bf16)
            nc.vector.memset(ones_bf[:], 1.0)
            nc.vector.memset(T_sb[:], 0.0)

            def build_T_row(jt, v_ap, s_col_ap, d_col_ap):
                """v [128,64] f32, s/d [128,1] f32 -> T_sb[:, jt, :]."""
                nc.vector.tensor_copy(T_sb[:, jt, 0:1], ones_bf[:])
                nc.vector.tensor_copy(T_sb[:, jt, 1:65], v_ap)
                nc.vector.tensor_copy(
                    T_sb[:, jt, 66:68].bitcast(f32), s_col_ap)
                nc.vector.tensor_copy(
                    T_sb[:, jt, 68:70].bitcast(f32), d_col_ap)

            # ================= layer 1 pre =================
            NHALF = 2
            with tc.tile_pool(name="l1", bufs=1) as l1p:
                W1_sb = l1p.tile([128, c_.F // 128, c_.H], f32)
                nc.sync.dma_start(
                    W1_sb[:], W_d[1][:].rearrange("(k p) n -> p k n", p=128))
                HN = c_.NLOC // NHALF
                for half in range(NHALF):
                    featT_sb = l1p.tile([128, c_.F // 128, HN], f32,
                                        tag="featT", name=f"featT_{half}")
                    nc.sync.dma_start(
                        featT_sb[:],
                        featT[:, half * HN:(half + 1) * HN]
                        .rearrange("(k p) n -> p k n", p=128))
                    for jt in range(half * (c_.NTILES // NHALF),
                                    (half + 1) * (c_.NTILES // NHALF)):
                        jl = jt - half * (c_.NTILES // NHALF)
                        ph = psum.tile([128, c_.H], f32, tag="ps_mm", bufs=2)
                        for k in range(c_.F // 128):
                            nc.tensor.matmul(
                                ph[:], featT_sb[:, k, bass.ts(jl, 128)],
                                W1_sb[:, k, :], start=(k == 0),
                                stop=(k == c_.F // 128 - 1))
                        h_t = evacp.tile([128, c_.H], f32, tag="h_t")
                        nc.vector.tensor_copy(h_t[:], ph[:])
                        nc.vector.tensor_copy(x_sb[:, jt, :], h_t[:])
                        s_c = evacp.tile([128, 1], f32, tag="s_c")
                        d_c = evacp.tile([128, 1], f32, tag="d_c")
                        tmp = evacp.tile([128, c_.H], f32, tag="tt_tmp")
                        nc.vector.tensor_tensor(tmp[:], h_t[:], sm["cs1"][:], op=ALU.mult)
                        nc.vector.tensor_reduce(s_c[:], tmp[:], axis=mybir.AxisListType.X,
                                                op=ALU.add)
                        nc.vector.tensor_tensor(tmp[:], h_t[:], sm["cd1"][:], op=ALU.mult)
                        nc.vector.tensor_reduce(d_c[:], tmp[:], axis=mybir.AxisListType.X,
                                                op=ALU.add)
                        build_T_row(jt, h_t[:], s_c[:], d_c[:])

            # ================= per layer =================
            for layer in (1, 2, 3, 4):
                # ---- write T_local + d_table, AllGather ----
                if layer > 1:
                    for jt in range(c_.NTILES):
                        s_c = evacp.tile([128, 1], f32, tag="s_c")
                        d_c = evacp.tile([128, 1], f32, tag="d_c")
                        tmp = evacp.tile([128, c_.H], f32, tag="tt_tmp")
                        nc.vector.tensor_tensor(tmp[:], x_sb[:, jt, :],
                                                sm[f"cs{layer}"][:], op=ALU.mult)
                        nc.vector.tensor_reduce(s_c[:], tmp[:],
                                                axis=mybir.AxisListType.X, op=ALU.add)
                        nc.vector.tensor_tensor(tmp[:], x_sb[:, jt, :],
                                                sm[f"cd{layer}"][:], op=ALU.mult)
                        nc.vector.tensor_reduce(d_c[:], tmp[:],
                                                axis=mybir.AxisListType.X, op=ALU.add)
                        build_T_row(jt, x_sb[:, jt, :], s_c[:], d_c[:])
                # T_local rows: row r = t*128 + p at offset r*128
                nc.sync.dma_start(
                    T_local[:].rearrange("(t p) e -> p t e", p=128), T_sb[:])
                nc.gpsimd.collective_compute(
                    "AllGather", mybir.AluOpType.bypass,
                    replica_groups=[list(range(c_.NCORES))],
                    ins=[T_local[:].opt()], outs=[T_full[:].opt()])

                dout = c_.F if layer == 4 else c_.H
                # ---- ranges ----
                pi = 0
                for r in range(c_.NRANGES):
                    ps_t = {}
                    for t in range(r * c_.TR, (r + 1) * c_.TR):
                        ps_t[t] = psum.tile([128, 65], f32, tag="ps_agg", name=f"ps_agg_{t}", bufs=5)
                    written = set()
                    for ch in range(c_.NCHUNK):
                        slot0, nsl, ch_, g0, ng = pieces[pi]
                        pi += 1
                        assert ch_ == ch
                        cols = nsl // 128
                        # per-group indirect gathers into piece buffers
                        G = work.tile([128, cols, 128], bf16, tag="G")
                        D = work.tile([128, cols, 128], bf16, tag="D")
                        for gi in range(ng):
                            g = g0 + gi
                            nc.gpsimd.indirect_dma_start(
                                out=G[:, gi, :], out_offset=None, in_=T_full[:],
                                in_offset=bass.IndirectOffsetOnAxis(
                                    ap=idx_hs_sb[:, g:g + 1], axis=0))
                            nc.gpsimd.indirect_dma_start(
                                out=D[:, gi, :], out_offset=None, in_=T_full[:],
                                in_offset=bass.IndirectOffsetOnAxis(
                                    ap=idx_d_sb[:, g:g + 1], axis=0))
                        # per-edge scalars
                        u = work.tile([128, cols], f32, tag="u")
                        nc.vector.tensor_tensor(
                            u[:], G[:, :, 66:68].bitcast(f32).squeeze(-1),
                            D[:, :, 68:70].bitcast(f32).squeeze(-1), op=ALU.add)
                        e_t = work.tile([128, cols], f32, tag="e")
                        nc.vector.scalar_tensor_tensor(
                            e_t[:], u[:], c_.SLOPE, u[:],
                            op0=ALU.mult, op1=ALU.max)
                        p_bf = work.tile([128, cols], bf16, tag="p")
                        nc.scalar.activation(p_bf[:], e_t[:], AF.Exp)
                        # onehot * p
                        ohw = work.tile([128, cols, 128], bf16, tag="ohw")
                        nc.vector.tensor_tensor(
                            ohw[:],
                            dst_rel_sb[:, g0:g0 + ng].unsqueeze(-1)
                            .to_broadcast([128, cols, 128]),
                            iota_bf[:].unsqueeze(1).to_broadcast([128, cols, 128]),
                            op=ALU.is_equal)
                        nc.vector.tensor_tensor(
                            ohw[:], ohw[:],
                            p_bf[:].unsqueeze(-1).to_broadcast([128, cols, 128]),
                            op=ALU.mult)
                        # matmuls
                        for gi in range(ng):
                            g = g0 + gi
                            t = int(group_t[g])
                            first = t not in written
                            written.add(t)
                            last = (ch == c_.NCHUNK - 1) and (
                                gi + 1 == ng or int(group_t[g0 + gi + 1]) != t)
                            nc.tensor.matmul(
                                ps_t[t][:], ohw[:, gi, :], G[:, gi, 0:65],
                                start=first, stop=last)
                    # ---- evacuate tiles of this range ----
                    for t in range(r * c_.TR, (r + 1) * c_.TR):
                        jt = t
                        ps = ps_t[t]
                        dtmp = evacp.tile([128, 1], f32, tag="dtmp")
                        nc.vector.tensor_scalar_add(dtmp[:], ps[:, 0:1], 1e-16)
                        nc.vector.reciprocal(den_sb[:, jt:jt + 1], dtmp[:])
                        acc = evacp.tile([128, 64], f32, tag="acc")
                        nc.scalar.activation(acc[:], ps[:, 1:65], AF.Copy)
                        if layer == 1:
                            xo = evacp.tile([128, c_.H], f32, tag="xo")
                            nc.vector.tensor_scalar_mul(
                                xo[:], acc[:], den_sb[:, jt:jt + 1])
                            nc.vector.tensor_tensor(xo[:], xo[:], sm["b1"][:],
                                                    op=ALU.add)
                            nc.scalar.activation(xo[:], xo[:], AF.Relu)
                            nc.vector.tensor_tensor(xo[:], xo[:], sm["scale1"][:],
                                                    op=ALU.mult)
                            nc.vector.tensor_tensor(x_sb[:, jt, :], xo[:],
                                                    sm["shift1"][:], op=ALU.add)
                        else:
                            ptr = psum.tile([64, 128], f32, tag="ps_tr", bufs=1)
                            nc.tensor.transpose(ptr[:], acc[:], ident[:])
                            accT = evacp.tile([64, 128], f32, tag="accT")
                            nc.vector.tensor_copy(accT[:], ptr[:])
                            px = psum.tile([128, dout], f32, tag="ps_mm", bufs=2)
                            nc.tensor.matmul(px[:], accT[:], W_sb[layer][:],
                                             start=True, stop=True)
                            xo = evacp.tile([128, dout], f32, tag="xo")
                            nc.vector.tensor_scalar_mul(
                                xo[:], px[:], den_sb[:, jt:jt + 1])
                            nc.vector.tensor_tensor(
                                xo[:], xo[:],
                                sm[f"b{layer}"][:, 0:dout], op=ALU.add)
                            nc.scalar.activation(xo[:], xo[:], AF.Relu)
                            if layer < 4:
                                nc.vector.tensor_tensor(
                                    xo[:], xo[:], sm[f"scale{layer}"][:],
                                    op=ALU.mult)
                                nc.vector.tensor_tensor(
                                    x_sb[:, jt, :], xo[:], sm[f"shift{layer}"][:],
                                    op=ALU.add)
                            else:
                                nc.sync.dma_start(
                                    out_d[jt * 128:(jt + 1) * 128, :], xo[:])
    nc.compile()
    return nc


def kernel(**inputs):
    cfg = FULL
    plan = build_plan(np.asarray(inputs["edge_index"]), cfg)
    in_maps = build_host_inputs(plan, cfg, inputs)
    nc = build_bass(plan, cfg)
    from concourse.bass_utils import run_bass_kernel_spmd
    res = run_bass_kernel_spmd(nc, in_maps, core_ids=list(range(cfg.NCORES)))
    outs = [res.results[c]["out"] for c in range(cfg.NCORES)]
    full = np.concatenate(outs, axis=0)[:cfg.N]
    return full.astype(np.float32)

